# revision 1
# baseline (speedup 1.0000x reference)
"""Trainium2 kernel for nn_MiddleHeadLayer: 2-layer tanh MLP + row-dot + sigmoid.

    inner = tanh(batch @ W1.T + b1)        batch [N, 1024], W1 [4096, 1024]
    wx    = tanh(inner @ W2.T + b2)        W2 [1024, 4096]
    out   = sigmoid(sum(wx * batch, -1))   [N]

Data-parallel over 8 NeuronCores: each core handles N/8 = 2048 rows;
weights replicated, resident in SBUF as fp16 (f32 weights do not fit in
24MB SBUF; fp16 matmuls run at full PE rate and keep absmax error ~3e-3).

Per-core dataflow, in blocks of R=256 rows:
  phase 1: innerT[dff, rows] = tanh(W1T.T @ batchT + b1) — stationary W1T
           chunks [128,128], moving batchT [128, R], fp16 in / f32 PSUM,
           ACT applies the per-partition (d_ff) bias and writes fp16.
  phase 2: wx[rows, dmodel] = tanh(innerT.T @ W2T + b2) — stationary innerT
           chunks, moving W2T [128, 512]. b2 (free-dim bias) is folded in
           as a rank-1 ones x b2 matmul into the same PSUM group.
  dot:     z[rows] = sum(wx * batch_f32) via fused DVE tensor_tensor_reduce
           along the free dim; sigmoid once at the end on all z columns.
"""

from contextlib import ExitStack

import numpy as np
import orjson

import concourse.bass as bass
import concourse.tile as tile
from concourse import mybir
from concourse import bass_utils

D_MODEL = 1024
D_FF = 4096
N_TOTAL = 16384
N_CORES = 8
NC_ROWS = N_TOTAL // N_CORES          # 2048 rows per core
R = 256                               # row-block size
N_BLOCKS = NC_ROWS // R               # 8
K1 = D_MODEL // 128                   # 8 contraction chunks for matmul1
M1 = D_FF // 128                      # 32 d_ff chunks
RG = R // 128                         # row groups per block
NH = D_MODEL // 512                   # d_model halves for phase 2
F16 = mybir.dt.float16
F32 = mybir.dt.float32


# ---------------------------------------------------------------------------
# This walrus build rejects >2 sem waits on a single instruction, while Tile's
# wait assignment freely attaches more (e.g. the exit drain gets one wait per
# outstanding logical proc). Legalize at the BIR-JSON level: hoist excess
# waits onto EventSemaphore instructions inserted directly before the
# offending instruction on the same engine stream (identical semantics).
MAX_WAITS = 1


def _legalize_sync_waits(bir: dict) -> dict:
    ctr = 0
    for fn in bir.get("functions", []):
        for blk in fn.get("blocks", []):
            insts = blk.get("instructions")
            if not insts:
                continue
            out = []
            changed = False
            for inst in insts:
                si = inst.get("sync_info")
                ow = (si or {}).get("on_wait") or []
                limit = 2 if inst.get("opcode") == "EventSemaphore" else MAX_WAITS
                if len(ow) > limit:
                    changed = True
                    excess, keep = ow[:-limit], ow[-limit:]
                    for i in range(0, len(excess), MAX_WAITS):
                        ctr += 1
                        out.append({
                            "debug": inst.get("debug"),
                            "engine": inst["engine"],
                            "ins": [],
                            "outs": [],
                            "name": f"legalwait-{ctr}",
                            "opcode": "EventSemaphore",
                            "sync_info": {
                                "on_update": [],
                                "on_wait": excess[i:i + MAX_WAITS],
                            },
                        })
                    si["on_wait"] = keep
                out.append(inst)
            if changed:
                blk["instructions"] = out
    return bir


_orig_to_json_bytes = bass.Bass.to_json_bytes


def _patched_to_json_bytes(self) -> bytes:
    return orjson.dumps(_legalize_sync_waits(orjson.loads(_orig_to_json_bytes(self))))


bass.Bass.to_json_bytes = _patched_to_json_bytes


def build_bass(n_blocks=N_BLOCKS):
    nc = bass.Bass("TRN2", target_bir_lowering=False, debug=False)

    w1t_d = nc.dram_tensor("w1t", [D_MODEL, D_FF], F16, kind="ExternalInput")
    w2t_d = nc.dram_tensor("w2t", [D_FF, D_MODEL], F16, kind="ExternalInput")
    b1_d = nc.dram_tensor("b1c", [128, M1], F32, kind="ExternalInput")
    b2_d = nc.dram_tensor("b2c", [1, D_MODEL], F16, kind="ExternalInput")
    ones_d = nc.dram_tensor("ones", [1, 128], F16, kind="ExternalInput")
    bt_d = nc.dram_tensor("batcht", [D_MODEL, NC_ROWS], F16, kind="ExternalInput")
    b_d = nc.dram_tensor("batch", [NC_ROWS, D_MODEL], F32, kind="ExternalInput")
    out_d = nc.dram_tensor("out", [NC_ROWS, 1], F32, kind="ExternalOutput")

    n_groups = n_blocks * RG
    W1CB = 4                       # w1t column blocks (of 1024 d_ff each)

    with tile.TileContext(nc) as tc, ExitStack() as ctx:
        wpool = ctx.enter_context(tc.tile_pool(name="weights", bufs=1))
        btpool = ctx.enter_context(tc.tile_pool(name="batchT", bufs=16))
        ipool = ctx.enter_context(tc.tile_pool(name="innerT", bufs=36))
        bfpool = ctx.enter_context(tc.tile_pool(name="batchf", bufs=4))
        wxpool = ctx.enter_context(tc.tile_pool(name="wx", bufs=4))
        spool = ctx.enter_context(tc.tile_pool(name="scratch", bufs=2))
        zpool = ctx.enter_context(tc.tile_pool(name="z", bufs=1))
        psum1 = ctx.enter_context(tc.tile_pool(name="psum1", bufs=3, space="PSUM"))
        psum2 = ctx.enter_context(tc.tile_pool(name="psum2", bufs=4, space="PSUM"))

        # DMA emission order = queue order: block-0 activations and the first
        # w1t column block go first so PE can start ~15us in; the remaining
        # weight bulk streams behind them.
        ones = wpool.tile([1, 128], F16, tag="ones")
        nc.sync.dma_start(ones[:], ones_d.ap()[:])

        bt0 = []
        for k in range(K1):
            t = btpool.tile([128, R], F16, tag="bt")
            nc.sync.dma_start(t[:], bt_d.ap()[k * 128:(k + 1) * 128, 0:R])
            bt0.append(t)

        CBW = D_FF // W1CB
        w1t = [[None] * W1CB for _ in range(K1)]
        for k in range(K1):
            t = wpool.tile([128, CBW], F16, tag=f"w1t{k}c0")
            nc.sync.dma_start(t[:], w1t_d.ap()[k * 128:(k + 1) * 128, 0:CBW])
            w1t[k][0] = t

        b1t = wpool.tile([128, M1], F32, tag="b1t")
        nc.sync.dma_start(b1t[:], b1_d.ap()[:])
        b2t = wpool.tile([1, D_MODEL], F16, tag="b2t")
        nc.sync.dma_start(b2t[:], b2_d.ap()[:])

        # rest of W1T column blocks
        for cb in range(1, W1CB):
            for k in range(K1):
                t = wpool.tile([128, CBW], F16, tag=f"w1t{k}c{cb}")
                nc.sync.dma_start(
                    t[:], w1t_d.ap()[k * 128:(k + 1) * 128, cb * CBW:(cb + 1) * CBW]
                )
                w1t[k][cb] = t
        # W2T chunks (first needed ~45us in, at phase 2 of block 0)
        w2t = []
        for m in range(M1):
            t = wpool.tile([128, D_MODEL], F16, tag=f"w2t{m}")
            nc.sync.dma_start(t[:], w2t_d.ap()[m * 128:(m + 1) * 128, :])
            w2t.append(t)

        z_all = zpool.tile([128, n_groups], F32)
        sig = zpool.tile([128, n_groups], F32, tag="sig")

        for b in range(n_blocks):
            # batchT chunks for this row block
            if b == 0:
                bt = bt0
            else:
                bt = []
                for k in range(K1):
                    t = btpool.tile([128, R], F16, tag="bt")
                    nc.sync.dma_start(
                        t[:], bt_d.ap()[k * 128:(k + 1) * 128, b * R:(b + 1) * R]
                    )
                    bt.append(t)

            # phase 1: innerT chunks [128 dff, R rows]
            it = []
            for m in range(M1):
                cb, mo = divmod(m, CBW // 128)
                ps = psum1.tile([128, R], F32)
                for k in range(K1):
                    nc.tensor.matmul(
                        ps[:],
                        w1t[k][cb][:, mo * 128:(mo + 1) * 128],
                        bt[k][:],
                        start=(k == 0),
                        stop=(k == K1 - 1),
                    )
                t = ipool.tile([128, R], F16, tag="it")
                nc.scalar.activation(
                    t[:], ps[:], mybir.ActivationFunctionType.Tanh,
                    bias=b1t[:, m:m + 1],
                )
                it.append(t)

            # phase 2 + row-dot per 128-row group
            for rg in range(RG):
                g = b * RG + rg
                bf = bfpool.tile([128, D_MODEL], F32, tag="bf")
                nc.sync.dma_start(
                    bf[:], b_d.ap()[g * 128:(g + 1) * 128, :]
                )
                wx = wxpool.tile([128, D_MODEL], F32, tag="wx")
                for h in range(NH):
                    ps2 = psum2.tile([128, 512], F32)
                    for m in range(M1):
                        nc.tensor.matmul(
                            ps2[:],
                            it[m][:, rg * 128:(rg + 1) * 128],
                            w2t[m][:, h * 512:(h + 1) * 512],
                            start=(m == 0),
                            stop=False,
                        )
                    # b2 (free-dim bias) as a rank-1 ones x b2 accumulate,
                    # last so the group opener is a regular K=128 matmul
                    nc.tensor.matmul(
                        ps2[:],
                        ones[:],
                        b2t[:, h * 512:(h + 1) * 512],
                        start=False,
                        stop=True,
                    )
                    nc.scalar.activation(
                        wx[:, h * 512:(h + 1) * 512], ps2[:],
                        mybir.ActivationFunctionType.Tanh,
                    )
                # z[g] = sum(wx * batch) along d_model, fused mult+reduce on DVE
                scratch = spool.tile([128, D_MODEL], F32, tag="scr")
                nc.vector.scalar_tensor_tensor(
                    out=scratch[:],
                    in0=wx[:],
                    scalar=1.0,
                    in1=bf[:],
                    op0=mybir.AluOpType.mult,
                    op1=mybir.AluOpType.mult,
                    accum_out=z_all[:, g:g + 1],
                )
                nc.scalar.activation(
                    sig[:, g:g + 1], z_all[:, g:g + 1],
                    mybir.ActivationFunctionType.Sigmoid,
                )
                nc.sync.dma_start(
                    out_d.ap()[g * 128:(g + 1) * 128, :], sig[:, g:g + 1]
                )

    return nc


_CACHED = {}


def _get_nc(n_blocks=N_BLOCKS):
    if n_blocks not in _CACHED:
        _CACHED[n_blocks] = build_bass(n_blocks)
    return _CACHED[n_blocks]


def _prep_in_maps(batch, W1, b1, W2, b2):
    batch = np.ascontiguousarray(batch, dtype=np.float32)
    w1t = np.ascontiguousarray(W1.T, dtype=np.float16)      # [1024, 4096]
    w2t = np.ascontiguousarray(W2.T, dtype=np.float16)      # [4096, 1024]
    # b1 as [128, 32]: column m holds b1[m*128:(m+1)*128] (per-partition bias)
    b1c = np.ascontiguousarray(
        np.asarray(b1, dtype=np.float32).reshape(M1, 128).T
    )
    b2c = np.ascontiguousarray(b2, dtype=np.float16).reshape(1, D_MODEL)
    ones = np.ones((1, 128), dtype=np.float16)
    batcht = np.ascontiguousarray(batch.T.astype(np.float16))  # [1024, 16384]

    in_maps = []
    for c in range(N_CORES):
        r0, r1 = c * NC_ROWS, (c + 1) * NC_ROWS
        in_maps.append({
            "w1t": w1t,
            "w2t": w2t,
            "b1c": b1c,
            "b2c": b2c,
            "ones": ones,
            "batcht": np.ascontiguousarray(batcht[:, r0:r1]),
            "batch": np.ascontiguousarray(batch[r0:r1]),
        })
    return in_maps


def kernel(batch, W1, b1, W2, b2, _trace=False, _trace_kwargs=None):
    in_maps = _prep_in_maps(batch, W1, b1, W2, b2)
    nc = _get_nc()
    res = bass_utils.run_bass_kernel_spmd(
        nc, in_maps, core_ids=list(range(N_CORES)),
        trace=_trace, **(_trace_kwargs or {}),
    )
    out = np.concatenate([res.results[c]["out"][:, 0] for c in range(N_CORES)])
    if _trace:
        return out, res
    return out



# revision 4
# speedup vs baseline: 1.0878x; 1.0878x over previous
"""Trainium2 kernel for nn_MiddleHeadLayer: 2-layer tanh MLP + row-dot + sigmoid.

    inner = tanh(batch @ W1.T + b1)        batch [N, 1024], W1 [4096, 1024]
    wx    = tanh(inner @ W2.T + b2)        W2 [1024, 4096]
    out   = sigmoid(sum(wx * batch, -1))   [N]

Data-parallel over 8 NeuronCores: each core handles N/8 = 2048 rows;
weights replicated, resident in SBUF as fp16 (fp16 matmuls run at full PE
rate; absmax error ~4e-3, well inside the 2e-2 gate).

Per-core dataflow, in blocks of R=256 rows:
  phase 1: innerT[dff, rows] = tanh(W1T.T @ batchT + b1) — stationary W1T
           chunks [128,128], moving batchT [128, R], fp16 in / f32 PSUM,
           ACT applies the per-partition (d_ff) bias and writes fp16.
  phase 2: wx[rows, dmodel] = tanh(innerT.T @ W2T + b2) — stationary innerT
           chunks, moving W2T [128, 512]. b2 (free-dim bias) is added by a
           DVE tensor_tensor in-place on PSUM (saves 32 rank-1 PE matmuls).
  dot:     z[rows] = sum(wx * batch_f16) via fused DVE scalar_tensor_tensor
           along the free dim; ONE sigmoid + ONE output DMA at the end.

DMA strategy (the baseline's bottleneck): all inputs are pre-packed on the
host into [128, L]-shaped tensors whose per-partition lines are 4-8 KB, so
the whole kernel needs ~28 large DMAs instead of 163 small ones.  Each
dma_start costs ~650 ns of issue time on its engine queue, so issue is
split across three queues (sync: W1/W2 + output, gpsimd: batchT/batch,
scalar: small constants) and ordered just-in-time so the PE's first matmul
can start ~11 us in and never starves.  Eight zero-input warmup matmuls run
during the initial DMA wait to flip the PE HAM clock gate to full rate
before real work arrives.
"""

from contextlib import ExitStack

import numpy as np
import orjson

import concourse.bass as bass
import concourse.tile as tile
from concourse import mybir
from concourse import bass_utils

D_MODEL = 1024
D_FF = 4096
N_TOTAL = 16384
N_CORES = 8
NC_ROWS = N_TOTAL // N_CORES          # 2048 rows per core
R = 256                               # row-block size
N_BLOCKS = NC_ROWS // R               # 8
K1 = D_MODEL // 128                   # 8 contraction chunks for matmul1
M1 = D_FF // 128                      # 32 d_ff chunks
RG = R // 128                         # row groups per block (2)
NH = D_MODEL // 512                   # d_model halves for phase 2 (2)
N_GROUPS = N_BLOCKS * RG              # 16
F16 = mybir.dt.float16
F32 = mybir.dt.float32


# ---------------------------------------------------------------------------
# This walrus build rejects >2 sem waits on a single instruction, while Tile's
# wait assignment freely attaches more (e.g. the exit drain gets one wait per
# outstanding logical proc). Legalize at the BIR-JSON level: hoist excess
# waits onto EventSemaphore instructions inserted directly before the
# offending instruction on the same engine stream (identical semantics).
MAX_WAITS = 1


def _legalize_sync_waits(bir: dict) -> dict:
    ctr = 0
    for fn in bir.get("functions", []):
        for blk in fn.get("blocks", []):
            insts = blk.get("instructions")
            if not insts:
                continue
            out = []
            changed = False
            for inst in insts:
                si = inst.get("sync_info")
                ow = (si or {}).get("on_wait") or []
                limit = 2 if inst.get("opcode") == "EventSemaphore" else MAX_WAITS
                if len(ow) > limit:
                    changed = True
                    excess, keep = ow[:-limit], ow[-limit:]
                    for i in range(0, len(excess), MAX_WAITS):
                        ctr += 1
                        out.append({
                            "debug": inst.get("debug"),
                            "engine": inst["engine"],
                            "ins": [],
                            "outs": [],
                            "name": f"legalwait-{ctr}",
                            "opcode": "EventSemaphore",
                            "sync_info": {
                                "on_update": [],
                                "on_wait": excess[i:i + MAX_WAITS],
                            },
                        })
                    si["on_wait"] = keep
                out.append(inst)
            if changed:
                blk["instructions"] = out
    return bir


_orig_to_json_bytes = bass.Bass.to_json_bytes


def _patched_to_json_bytes(self) -> bytes:
    return orjson.dumps(_legalize_sync_waits(orjson.loads(_orig_to_json_bytes(self))))


bass.Bass.to_json_bytes = _patched_to_json_bytes


def build_bass(n_blocks=N_BLOCKS):
    nc = bass.Bass("TRN2", target_bir_lowering=False, debug=False)

    # Packed DRAM layouts (see _prep_in_maps for the exact packing):
    #  w1p[p, ((q*4 + mo4)*8 + k)*128 + j] = W1T[k*128+p, (q//2)*1024 + ((q%2)*4+mo4)*128 + j]
    #  w2p[p, (q*4 + ml)*1024 + c]         = W2T[(q*4+ml)*128 + p, c]
    #  btp[p, (b*8 + k)*256 + r]           = batchT[k*128+p, b*256 + r]
    #  bfp[p, g*1024 + c]                  = batch[g*128+p, c]            (fp16)
    w1p_d = nc.dram_tensor("w1p", [128, 8 * 4096], F16, kind="ExternalInput")
    w2p_d = nc.dram_tensor("w2p", [128, 8 * 4096], F16, kind="ExternalInput")
    btp_d = nc.dram_tensor("btp", [128, n_blocks * 8 * R], F16, kind="ExternalInput")
    bfp_d = nc.dram_tensor("bfp", [128, n_blocks * RG * D_MODEL], F16,
                           kind="ExternalInput")
    b1_d = nc.dram_tensor("b1c", [128, M1], F32, kind="ExternalInput")
    b2_d = nc.dram_tensor("b2r", [128, D_MODEL], F32, kind="ExternalInput")
    out_d = nc.dram_tensor("out", [128, n_blocks * RG], F32, kind="ExternalOutput")

    n_groups = n_blocks * RG

    with tile.TileContext(nc) as tc, ExitStack() as ctx:
        wpool = ctx.enter_context(tc.tile_pool(name="weights", bufs=1))
        btpool = ctx.enter_context(tc.tile_pool(name="batchT", bufs=2))
        bfpool = ctx.enter_context(tc.tile_pool(name="batchf", bufs=2))
        ipool = ctx.enter_context(tc.tile_pool(name="innerT", bufs=36))
        wxpool = ctx.enter_context(tc.tile_pool(name="wx", bufs=4))
        spool = ctx.enter_context(tc.tile_pool(name="scratch", bufs=2))
        zpool = ctx.enter_context(tc.tile_pool(name="z", bufs=1))
        psum1 = ctx.enter_context(tc.tile_pool(name="psum1", bufs=3, space="PSUM"))
        psum2 = ctx.enter_context(tc.tile_pool(name="psum2", bufs=4, space="PSUM"))
        psumw = ctx.enter_context(tc.tile_pool(name="psumw", bufs=1, space="PSUM"))

        # --- small constants on the scalar issue queue (needed by ~12 us) ---
        b1t = wpool.tile([128, M1], F32, tag="b1t")
        nc.scalar.dma_start(b1t[:], b1_d.ap()[:])
        b2r = wpool.tile([128, D_MODEL], F32, tag="b2r")
        nc.scalar.dma_start(b2r[:], b2_d.ap()[:])

        # --- PE warmup: ~3.4us of zero matmuls during the initial DMA wait
        # flips the HAM clock gate to 8/8 so real matmuls start at 2.4 GHz ---
        warm_s = wpool.tile([128, 128], F16, tag="warm_s")
        warm_m = wpool.tile([128, 512], F16, tag="warm_m")
        nc.vector.memset(warm_s[:], 0.0)
        nc.vector.memset(warm_m[:], 0.0)
        wps = psumw.tile([128, 512], F32)
        for _ in range(8):
            nc.tensor.matmul(wps[:], warm_s[:], warm_m[:], start=True, stop=True,
                             skip_group_check=True)

        # --- batch streams on the gpsimd issue queue, just-in-time order;
        # bufs=2 pools make later DMAs self-throttle behind buffer reuse ---
        bt_t, bf_t = [], []
        for b in range(n_blocks):
            t = btpool.tile([128, 8 * R], F16, tag="bt")
            nc.gpsimd.dma_start(t[:], btp_d.ap()[:, b * 8 * R:(b + 1) * 8 * R])
            bt_t.append(t)
            f = bfpool.tile([128, RG * D_MODEL], F16, tag="bf")
            nc.gpsimd.dma_start(
                f[:], bfp_d.ap()[:, b * RG * D_MODEL:(b + 1) * RG * D_MODEL])
            bf_t.append(f)

        # --- weight streams on the sync issue queue: W1 eighths first (block-0
        # phase 1 consumes them in order), then W2 eighths (phase 2 of block 0
        # starts ~40 us in) ---
        w1t = []
        for q in range(8):
            t = wpool.tile([128, 4096], F16, tag=f"w1q{q}")
            nc.sync.dma_start(t[:], w1p_d.ap()[:, q * 4096:(q + 1) * 4096])
            w1t.append(t)
        w2t = []
        for q in range(8):
            t = wpool.tile([128, 4096], F16, tag=f"w2q{q}")
            nc.sync.dma_start(t[:], w2p_d.ap()[:, q * 4096:(q + 1) * 4096])
            w2t.append(t)

        def w1s(m, k):
            # stationary [128, 128] for phase-1 (m, k)
            q, mo4 = divmod(m, 4)
            off = (mo4 * 8 + k) * 128
            return w1t[q][:, off:off + 128]

        def w2s(m, h):
            # moving [128, 512] for phase-2 (m, h)
            q, ml = divmod(m, 4)
            off = ml * 1024 + h * 512
            return w2t[q][:, off:off + 512]

        z_all = zpool.tile([128, n_groups], F32)
        sig = zpool.tile([128, n_groups], F32, tag="sig")

        for b in range(n_blocks):
            bt = bt_t[b]
            # phase 1: innerT chunks [128 dff, R rows]
            it = []
            for m in range(M1):
                ps = psum1.tile([128, R], F32)
                for k in range(K1):
                    nc.tensor.matmul(
                        ps[:],
                        w1s(m, k),
                        bt[:, k * R:(k + 1) * R],
                        start=(k == 0),
                        stop=(k == K1 - 1),
                    )
                t = ipool.tile([128, R], F16, tag="it")
                nc.scalar.activation(
                    t[:], ps[:], mybir.ActivationFunctionType.Tanh,
                    bias=b1t[:, m:m + 1],
                )
                it.append(t)

            # phase 2 + row-dot per 128-row group
            for rg in range(RG):
                g = b * RG + rg
                wx = wxpool.tile([128, D_MODEL], F16, tag="wx")
                for h in range(NH):
                    ps2 = psum2.tile([128, 512], F32)
                    for m in range(M1):
                        nc.tensor.matmul(
                            ps2[:],
                            it[m][:, rg * 128:(rg + 1) * 128],
                            w2s(m, h),
                            start=(m == 0),
                            stop=(m == M1 - 1),
                        )
                    # b2 (free-dim bias): DVE add in-place on PSUM
                    nc.vector.tensor_tensor(
                        ps2[:], ps2[:], b2r[:, h * 512:(h + 1) * 512],
                        mybir.AluOpType.add,
                    )
                    nc.scalar.activation(
                        wx[:, h * 512:(h + 1) * 512], ps2[:],
                        mybir.ActivationFunctionType.Tanh,
                    )
                # z[g] = sum(wx * batch) along d_model, fused mult+reduce on DVE
                scratch = spool.tile([128, D_MODEL], F16, tag="scr")
                nc.vector.scalar_tensor_tensor(
                    out=scratch[:],
                    in0=wx[:],
                    scalar=1.0,
                    in1=bf_t[b][:, rg * D_MODEL:(rg + 1) * D_MODEL],
                    op0=mybir.AluOpType.mult,
                    op1=mybir.AluOpType.mult,
                    accum_out=z_all[:, g:g + 1],
                )

        # one sigmoid + one output DMA; host untransposes [128, n_groups]
        nc.scalar.activation(
            sig[:], z_all[:], mybir.ActivationFunctionType.Sigmoid,
        )
        nc.sync.dma_start(out_d.ap()[:], sig[:])

    return nc


_CACHED = {}


def _get_nc(n_blocks=N_BLOCKS):
    if n_blocks not in _CACHED:
        _CACHED[n_blocks] = build_bass(n_blocks)
    return _CACHED[n_blocks]


def _prep_in_maps(batch, W1, b1, W2, b2):
    batch = np.ascontiguousarray(batch, dtype=np.float32)
    w1t = W1.T.astype(np.float16)                           # [1024, 4096]
    w2t = W2.T.astype(np.float16)                           # [4096, 1024]

    # w1p: [p, q, mo4, k, j] with m = q*4 + mo4 (cb = q//2, mo = ...)
    #   A[k, p, m, j] -> [p, m(=32), k, j] -> split m into (8, 4) -> pack
    A = w1t.reshape(K1, 128, M1, 128).transpose(1, 2, 0, 3)   # [p, m, k, j]
    w1p = np.ascontiguousarray(
        A.reshape(128, 8, 4, K1, 128).reshape(128, 8 * 4096))

    # w2p: [p, q, ml, c] with m = q*4 + ml
    C = w2t.reshape(M1, 128, D_MODEL).transpose(1, 0, 2)      # [p, m, c]
    w2p = np.ascontiguousarray(C.reshape(128, 8 * 4096))

    # b1 as [128, 32]: column m holds b1[m*128:(m+1)*128] (per-partition bias)
    b1c = np.ascontiguousarray(
        np.asarray(b1, dtype=np.float32).reshape(M1, 128).T)
    # b2 replicated across partitions for the DVE free-dim bias add
    b2r = np.ascontiguousarray(
        np.broadcast_to(np.asarray(b2, dtype=np.float32)[None, :],
                        (128, D_MODEL)))

    batcht = batch.T.astype(np.float16)                       # [1024, 16384]
    batch16 = batch.astype(np.float16)                        # [16384, 1024]

    in_maps = []
    for c in range(N_CORES):
        r0, r1 = c * NC_ROWS, (c + 1) * NC_ROWS
        # btp: [p, b, k, r]
        D = batcht[:, r0:r1].reshape(K1, 128, N_BLOCKS, R).transpose(1, 2, 0, 3)
        btp = np.ascontiguousarray(D.reshape(128, N_BLOCKS * 8 * R))
        # bfp: [p, g, c]
        E = batch16[r0:r1].reshape(N_GROUPS, 128, D_MODEL).transpose(1, 0, 2)
        bfp = np.ascontiguousarray(E.reshape(128, N_GROUPS * D_MODEL))
        in_maps.append({
            "w1p": w1p,
            "w2p": w2p,
            "b1c": b1c,
            "b2r": b2r,
            "btp": btp,
            "bfp": bfp,
        })
    return in_maps


def kernel(batch, W1, b1, W2, b2, _trace=False, _trace_kwargs=None):
    in_maps = _prep_in_maps(batch, W1, b1, W2, b2)
    nc = _get_nc()
    res = bass_utils.run_bass_kernel_spmd(
        nc, in_maps, core_ids=list(range(N_CORES)),
        trace=_trace, **(_trace_kwargs or {}),
    )
    # out[p, g] holds row g*128+p of the core's 2048 rows
    out = np.concatenate([
        np.ascontiguousarray(res.results[c]["out"].T).reshape(-1)
        for c in range(N_CORES)
    ])
    if _trace:
        return out, res
    return out


# revision 9
# speedup vs baseline: 1.0965x; 1.0081x over previous
"""Trainium2 kernel for nn_MiddleHeadLayer: 2-layer tanh MLP + row-dot + sigmoid.

    inner = tanh(batch @ W1.T + b1)        batch [N, 1024], W1 [4096, 1024]
    wx    = tanh(inner @ W2.T + b2)        W2 [1024, 4096]
    out   = sigmoid(sum(wx * batch, -1))   [N]

Data-parallel over 8 NeuronCores: each core handles N/8 = 2048 rows;
weights replicated, resident in SBUF as fp16 (fp16 matmuls run at full PE
rate; absmax error ~4e-3, well inside the 2e-2 gate).

Per-core dataflow, in blocks of R=256 rows:
  phase 1: innerT[dff, rows] = tanh(W1T.T @ batchT + b1) — stationary W1T
           chunks [128,128], moving batchT [128, R], fp16 in / f32 PSUM,
           ACT applies the per-partition (d_ff) bias and writes fp16.
  phase 2: wx[rows, dmodel] = tanh(innerT.T @ W2T + b2) — stationary innerT
           chunks, moving W2T [128, 512]. b2 (free-dim bias) is added by a
           DVE tensor_tensor in-place on PSUM (saves 32 rank-1 PE matmuls).
  dot:     z[rows] = sum(wx * batch_f16) via fused DVE scalar_tensor_tensor
           along the free dim; ONE sigmoid + ONE output DMA at the end.

DMA strategy (the baseline's bottleneck): all inputs are pre-packed on the
host into [128, L]-shaped tensors whose per-partition lines are 4-8 KB, so
the whole kernel needs ~28 large DMAs instead of 163 small ones.  Each
dma_start costs ~650 ns of issue time on its engine queue, so issue is
split across three queues (sync: W1/W2 + output, gpsimd: batchT/batch,
scalar: small constants) and ordered just-in-time so the PE's first matmul
can start ~11 us in and never starves.  Eight zero-input warmup matmuls run
during the initial DMA wait to flip the PE HAM clock gate to full rate
before real work arrives.
"""

from contextlib import ExitStack

import numpy as np
import orjson

import concourse.bass as bass
import concourse.tile as tile
from concourse import mybir
from concourse import bass_utils

D_MODEL = 1024
D_FF = 4096
N_TOTAL = 16384
N_CORES = 8
NC_ROWS = N_TOTAL // N_CORES          # 2048 rows per core
R = 256                               # row-block size
N_BLOCKS = NC_ROWS // R               # 8
K1 = D_MODEL // 128                   # 8 contraction chunks for matmul1
M1 = D_FF // 128                      # 32 d_ff chunks
RG = R // 128                         # row groups per block (2)
NH = D_MODEL // 512                   # d_model halves for phase 2 (2)
N_GROUPS = N_BLOCKS * RG              # 16
F16 = mybir.dt.float16
F32 = mybir.dt.float32


# ---------------------------------------------------------------------------
# This walrus build rejects >2 sem waits on a single instruction, while Tile's
# wait assignment freely attaches more (e.g. the exit drain gets one wait per
# outstanding logical proc). Legalize at the BIR-JSON level: hoist excess
# waits onto EventSemaphore instructions inserted directly before the
# offending instruction on the same engine stream (identical semantics).
MAX_WAITS = 1


def _legalize_sync_waits(bir: dict) -> dict:
    ctr = 0
    for fn in bir.get("functions", []):
        for blk in fn.get("blocks", []):
            insts = blk.get("instructions")
            if not insts:
                continue
            out = []
            changed = False
            for inst in insts:
                si = inst.get("sync_info")
                ow = (si or {}).get("on_wait") or []
                limit = 2 if inst.get("opcode") == "EventSemaphore" else MAX_WAITS
                if len(ow) > limit:
                    changed = True
                    excess, keep = ow[:-limit], ow[-limit:]
                    for i in range(0, len(excess), MAX_WAITS):
                        ctr += 1
                        out.append({
                            "debug": inst.get("debug"),
                            "engine": inst["engine"],
                            "ins": [],
                            "outs": [],
                            "name": f"legalwait-{ctr}",
                            "opcode": "EventSemaphore",
                            "sync_info": {
                                "on_update": [],
                                "on_wait": excess[i:i + MAX_WAITS],
                            },
                        })
                    si["on_wait"] = keep
                out.append(inst)
            if changed:
                blk["instructions"] = out
    return bir


_orig_to_json_bytes = bass.Bass.to_json_bytes


def _patched_to_json_bytes(self) -> bytes:
    return orjson.dumps(_legalize_sync_waits(orjson.loads(_orig_to_json_bytes(self))))


bass.Bass.to_json_bytes = _patched_to_json_bytes


def build_bass(n_blocks=N_BLOCKS):
    nc = bass.Bass("TRN2", target_bir_lowering=False, debug=False)

    # Packed DRAM layouts (see _prep_in_maps for the exact packing):
    #  w1p[p, (q*16 + mo2*8 + k)*128 + j] = W1T[k*128+p, (q*2+mo2)*128 + j]
    #  w2p[p, (q*4 + ml)*1024 + c]         = W2T[(q*4+ml)*128 + p, c]
    #  btp[p, (b*8 + k)*256 + r]           = batchT[k*128+p, b*256 + r]
    #  bfp[p, g*1024 + c]                  = batch[g*128+p, c]            (fp16)
    w1p_d = nc.dram_tensor("w1p", [128, 8 * 4096], F16, kind="ExternalInput")
    w2p_d = nc.dram_tensor("w2p", [128, 8 * 4096], F16, kind="ExternalInput")
    btp_d = nc.dram_tensor("btp", [128, n_blocks * 8 * R], F16, kind="ExternalInput")
    bfp_d = nc.dram_tensor("bfp", [128, n_blocks * RG * D_MODEL], F16,
                           kind="ExternalInput")
    b1_d = nc.dram_tensor("b1c", [128, M1], F32, kind="ExternalInput")
    b2_d = nc.dram_tensor("b2r", [128, D_MODEL], F32, kind="ExternalInput")
    out_d = nc.dram_tensor("out", [128, n_blocks * RG], F32, kind="ExternalOutput")

    n_groups = n_blocks * RG

    with tile.TileContext(nc) as tc, ExitStack() as ctx:
        wpool = ctx.enter_context(tc.tile_pool(name="weights", bufs=1))
        btpool = ctx.enter_context(tc.tile_pool(name="batchT", bufs=1))
        bfpool = ctx.enter_context(tc.tile_pool(name="batchf", bufs=1))
        ipool = ctx.enter_context(tc.tile_pool(name="innerT", bufs=36))
        wxpool = ctx.enter_context(tc.tile_pool(name="wx", bufs=4))
        spool = ctx.enter_context(tc.tile_pool(name="scratch", bufs=2))
        zpool = ctx.enter_context(tc.tile_pool(name="z", bufs=1))
        psum1 = ctx.enter_context(tc.tile_pool(name="psum1", bufs=3, space="PSUM"))
        psum2 = ctx.enter_context(tc.tile_pool(name="psum2", bufs=4, space="PSUM"))
        psumw = ctx.enter_context(tc.tile_pool(name="psumw", bufs=1, space="PSUM"))

        # --- small constants on the scalar issue queue (needed by ~12 us) ---
        b1t = wpool.tile([128, M1], F32, tag="b1t")
        nc.scalar.dma_start(b1t[:], b1_d.ap()[:])

        # --- PE warmup: zero matmuls during the initial DMA wait flip the
        # HAM clock gate to 8/8 and bridge until real data lands (~13 us) ---
        warm_s = wpool.tile([128, 128], F16, tag="warm_s")
        warm_m = wpool.tile([128, 512], F16, tag="warm_m")
        nc.vector.memset(warm_s[:], 0.0)
        nc.vector.memset(warm_m[:], 0.0)
        wps = psumw.tile([128, 512], F32)
        for _ in range(15):
            nc.tensor.matmul(wps[:], warm_s[:], warm_m[:], start=True, stop=True,
                             skip_group_check=True)

        # --- batch streams on the gpsimd issue queue, just-in-time order;
        # bufs=1 pools make later DMAs self-throttle behind buffer reuse so
        # their ring traffic never competes with the critical weight stream ---
        bt_t, bf_t = [], []

        def emit_bt(b):
            t = btpool.tile([128, 8 * R], F16, tag="bt")
            nc.gpsimd.dma_start(t[:], btp_d.ap()[:, b * 8 * R:(b + 1) * 8 * R])
            bt_t.append(t)

        def emit_bf(b):
            f = bfpool.tile([128, RG * D_MODEL], F16, tag="bf")
            nc.gpsimd.dma_start(
                f[:], bfp_d.ap()[:, b * RG * D_MODEL:(b + 1) * RG * D_MODEL])
            bf_t.append(f)

        emit_bt(0)
        if n_blocks > 1:
            emit_bt(1)
        emit_bf(0)

        # --- weight streams on the sync issue queue: W1 sixteenths first
        # (block-0 phase 1 consumes them in order; fine split so the first
        # matmul can start ~13 us in), then b2, then W2 eighths (phase 2 of
        # block 0 starts ~40 us in) ---
        w1t = []
        for q in range(16):
            t = wpool.tile([128, 2048], F16, tag=f"w1q{q}")
            nc.sync.dma_start(t[:], w1p_d.ap()[:, q * 2048:(q + 1) * 2048])
            w1t.append(t)
        b2r = wpool.tile([128, D_MODEL], F32, tag="b2r")
        nc.sync.dma_start(b2r[:], b2_d.ap()[:])
        w2t = []
        for q in range(8):
            t = wpool.tile([128, 4096], F16, tag=f"w2q{q}")
            nc.sync.dma_start(t[:], w2p_d.ap()[:, q * 4096:(q + 1) * 4096])
            w2t.append(t)

        def w1s(m, k):
            # stationary [128, 128] for phase-1 (m, k)
            q, mo2 = divmod(m, 2)
            off = (mo2 * 8 + k) * 128
            return w1t[q][:, off:off + 128]

        def w2s(m, h):
            # moving [128, 512] for phase-2 (m, h)
            q, ml = divmod(m, 4)
            off = ml * 1024 + h * 512
            return w2t[q][:, off:off + 512]

        z_all = zpool.tile([128, n_groups], F32)
        sig = zpool.tile([128, n_groups], F32, tag="sig")

        for b in range(n_blocks):
            # just-in-time prefetch of later batch blocks (queue-order +
            # bufs=1 waits give them exactly-when-needed ring bandwidth)
            if b + 2 < n_blocks:
                emit_bt(b + 2)
            if 0 < b + 1 < n_blocks:
                emit_bf(b + 1)
            bt = bt_t[b]
            # phase 1: innerT chunks [128 dff, R rows]
            it = []
            for m in range(M1):
                ps = psum1.tile([128, R], F32)
                for k in range(K1):
                    nc.tensor.matmul(
                        ps[:],
                        w1s(m, k),
                        bt[:, k * R:(k + 1) * R],
                        start=(k == 0),
                        stop=(k == K1 - 1),
                    )
                t = ipool.tile([128, R], F16, tag="it")
                nc.scalar.activation(
                    t[:], ps[:], mybir.ActivationFunctionType.Tanh,
                    bias=b1t[:, m:m + 1],
                )
                it.append(t)

            # phase 2 + row-dot per 128-row group
            for rg in range(RG):
                g = b * RG + rg
                wx = wxpool.tile([128, D_MODEL], F16, tag="wx")
                for h in range(NH):
                    ps2 = psum2.tile([128, 512], F32)
                    for m in range(M1):
                        nc.tensor.matmul(
                            ps2[:],
                            it[m][:, rg * 128:(rg + 1) * 128],
                            w2s(m, h),
                            start=(m == 0),
                            stop=(m == M1 - 1),
                        )
                    # b2 (free-dim bias): DVE add in-place on PSUM
                    nc.vector.tensor_tensor(
                        ps2[:], ps2[:], b2r[:, h * 512:(h + 1) * 512],
                        mybir.AluOpType.add,
                    )
                    nc.scalar.activation(
                        wx[:, h * 512:(h + 1) * 512], ps2[:],
                        mybir.ActivationFunctionType.Tanh,
                    )
                # z[g] = sum(wx * batch) along d_model, fused mult+reduce on DVE
                scratch = spool.tile([128, D_MODEL], F16, tag="scr")
                nc.vector.scalar_tensor_tensor(
                    out=scratch[:],
                    in0=wx[:],
                    scalar=1.0,
                    in1=bf_t[b][:, rg * D_MODEL:(rg + 1) * D_MODEL],
                    op0=mybir.AluOpType.mult,
                    op1=mybir.AluOpType.mult,
                    accum_out=z_all[:, g:g + 1],
                )

        # one sigmoid + one output DMA; host untransposes [128, n_groups]
        nc.scalar.activation(
            sig[:], z_all[:], mybir.ActivationFunctionType.Sigmoid,
        )
        nc.sync.dma_start(out_d.ap()[:], sig[:])

    return nc


_CACHED = {}


def _get_nc(n_blocks=N_BLOCKS):
    if n_blocks not in _CACHED:
        _CACHED[n_blocks] = build_bass(n_blocks)
    return _CACHED[n_blocks]


def _prep_in_maps(batch, W1, b1, W2, b2):
    batch = np.ascontiguousarray(batch, dtype=np.float32)
    w1t = W1.T.astype(np.float16)                           # [1024, 4096]
    w2t = W2.T.astype(np.float16)                           # [4096, 1024]

    # w1p: [p, q, mo2, k, j] with m = q*2 + mo2
    #   A[k, p, m, j] -> [p, m(=32), k, j] -> split m into (16, 2) -> pack
    A = w1t.reshape(K1, 128, M1, 128).transpose(1, 2, 0, 3)   # [p, m, k, j]
    w1p = np.ascontiguousarray(
        A.reshape(128, 16, 2, K1, 128).reshape(128, 8 * 4096))

    # w2p: [p, q, ml, c] with m = q*4 + ml
    C = w2t.reshape(M1, 128, D_MODEL).transpose(1, 0, 2)      # [p, m, c]
    w2p = np.ascontiguousarray(C.reshape(128, 8 * 4096))

    # b1 as [128, 32]: column m holds b1[m*128:(m+1)*128] (per-partition bias)
    b1c = np.ascontiguousarray(
        np.asarray(b1, dtype=np.float32).reshape(M1, 128).T)
    # b2 replicated across partitions for the DVE free-dim bias add
    b2r = np.ascontiguousarray(
        np.broadcast_to(np.asarray(b2, dtype=np.float32)[None, :],
                        (128, D_MODEL)))

    batcht = batch.T.astype(np.float16)                       # [1024, 16384]
    batch16 = batch.astype(np.float16)                        # [16384, 1024]

    in_maps = []
    for c in range(N_CORES):
        r0, r1 = c * NC_ROWS, (c + 1) * NC_ROWS
        # btp: [p, b, k, r]
        D = batcht[:, r0:r1].reshape(K1, 128, N_BLOCKS, R).transpose(1, 2, 0, 3)
        btp = np.ascontiguousarray(D.reshape(128, N_BLOCKS * 8 * R))
        # bfp: [p, g, c]
        E = batch16[r0:r1].reshape(N_GROUPS, 128, D_MODEL).transpose(1, 0, 2)
        bfp = np.ascontiguousarray(E.reshape(128, N_GROUPS * D_MODEL))
        in_maps.append({
            "w1p": w1p,
            "w2p": w2p,
            "b1c": b1c,
            "b2r": b2r,
            "btp": btp,
            "bfp": bfp,
        })
    return in_maps


def kernel(batch, W1, b1, W2, b2, _trace=False, _trace_kwargs=None):
    in_maps = _prep_in_maps(batch, W1, b1, W2, b2)
    nc = _get_nc()
    res = bass_utils.run_bass_kernel_spmd(
        nc, in_maps, core_ids=list(range(N_CORES)),
        trace=_trace, **(_trace_kwargs or {}),
    )
    # out[p, g] holds row g*128+p of the core's 2048 rows
    out = np.concatenate([
        np.ascontiguousarray(res.results[c]["out"].T).reshape(-1)
        for c in range(N_CORES)
    ])
    if _trace:
        return out, res
    return out


# revision 15
# speedup vs baseline: 1.1011x; 1.0041x over previous
"""Trainium2 kernel for nn_MiddleHeadLayer: 2-layer tanh MLP + row-dot + sigmoid.

    inner = tanh(batch @ W1.T + b1)        batch [N, 1024], W1 [4096, 1024]
    wx    = tanh(inner @ W2.T + b2)        W2 [1024, 4096]
    out   = sigmoid(sum(wx * batch, -1))   [N]

Data-parallel over 8 NeuronCores: each core handles N/8 = 2048 rows;
weights replicated, resident in SBUF as fp16 (fp16 matmuls run at full PE
rate; absmax error ~4e-3, well inside the 2e-2 gate).

Per-core dataflow, in blocks of R=256 rows:
  phase 1: innerT[dff, rows] = tanh(W1T.T @ batchT + b1) — stationary W1T
           chunks [128,128], moving batchT [128, R], fp16 in / f32 PSUM,
           ACT applies the per-partition (d_ff) bias and writes fp16.
  phase 2: wx[rows, dmodel] = tanh(innerT.T @ W2T + b2) — stationary innerT
           chunks, moving W2T [128, 512]. b2 (free-dim bias) is added by a
           DVE tensor_tensor in-place on PSUM (saves 32 rank-1 PE matmuls).
  dot:     z[rows] = sum(wx * batch_f16) via fused DVE scalar_tensor_tensor
           along the free dim; ONE sigmoid + ONE output DMA at the end.

DMA strategy (the baseline's bottleneck): all inputs are pre-packed on the
host into [128, L]-shaped tensors whose per-partition lines are 4-8 KB, so
the whole kernel needs ~28 large DMAs instead of 163 small ones.  Each
dma_start costs ~650 ns of issue time on its engine queue, so issue is
split across three queues (sync: W1/W2 + output, gpsimd: batchT/batch,
scalar: small constants) and ordered just-in-time so the PE's first matmul
can start ~11 us in and never starves.  Eight zero-input warmup matmuls run
during the initial DMA wait to flip the PE HAM clock gate to full rate
before real work arrives.
"""

from contextlib import ExitStack

import numpy as np
import orjson

import concourse.bass as bass
import concourse.tile as tile
from concourse import mybir
from concourse import bass_utils

D_MODEL = 1024
D_FF = 4096
N_TOTAL = 16384
N_CORES = 8
NC_ROWS = N_TOTAL // N_CORES          # 2048 rows per core
R = 256                               # row-block size
N_BLOCKS = NC_ROWS // R               # 8
K1 = D_MODEL // 128                   # 8 contraction chunks for matmul1
M1 = D_FF // 128                      # 32 d_ff chunks
RG = R // 128                         # row groups per block (2)
NH = D_MODEL // 512                   # d_model halves for phase 2 (2)
N_GROUPS = N_BLOCKS * RG              # 16
F16 = mybir.dt.float16
F32 = mybir.dt.float32


# ---------------------------------------------------------------------------
# This walrus build rejects >2 sem waits on a single instruction, while Tile's
# wait assignment freely attaches more (e.g. the exit drain gets one wait per
# outstanding logical proc). Legalize at the BIR-JSON level: hoist excess
# waits onto EventSemaphore instructions inserted directly before the
# offending instruction on the same engine stream (identical semantics).
MAX_WAITS = 1


def _legalize_sync_waits(bir: dict) -> dict:
    ctr = 0
    for fn in bir.get("functions", []):
        for blk in fn.get("blocks", []):
            insts = blk.get("instructions")
            if not insts:
                continue
            out = []
            changed = False
            for inst in insts:
                si = inst.get("sync_info")
                ow = (si or {}).get("on_wait") or []
                limit = 2 if inst.get("opcode") == "EventSemaphore" else MAX_WAITS
                if len(ow) > limit:
                    changed = True
                    excess, keep = ow[:-limit], ow[-limit:]
                    for i in range(0, len(excess), 2):
                        ctr += 1
                        out.append({
                            "debug": inst.get("debug"),
                            "engine": inst["engine"],
                            "ins": [],
                            "outs": [],
                            "name": f"legalwait-{ctr}",
                            "opcode": "EventSemaphore",
                            "sync_info": {
                                "on_update": [],
                                "on_wait": excess[i:i + 2],
                            },
                        })
                    si["on_wait"] = keep
                out.append(inst)
            if changed:
                blk["instructions"] = out
    return bir


_orig_to_json_bytes = bass.Bass.to_json_bytes


def _patched_to_json_bytes(self) -> bytes:
    return orjson.dumps(_legalize_sync_waits(orjson.loads(_orig_to_json_bytes(self))))


bass.Bass.to_json_bytes = _patched_to_json_bytes


def build_bass(n_blocks=N_BLOCKS):
    nc = bass.Bass("TRN2", target_bir_lowering=False, debug=False)

    # Packed DRAM layouts (see _prep_in_maps for the exact packing):
    #  w1p[p, (q*16 + mo2*8 + k)*128 + j] = W1T[k*128+p, (q*2+mo2)*128 + j]
    #  w2p[p, (q*4 + ml)*1024 + c]         = W2T[(q*4+ml)*128 + p, c]
    #  btp[p, (b*8 + k)*256 + r]           = batchT[k*128+p, b*256 + r]
    #  bfp[p, g*1024 + c]                  = batch[g*128+p, c]            (fp16)
    w1p_d = nc.dram_tensor("w1p", [128, 8 * 4096], F16, kind="ExternalInput")
    w2p_d = nc.dram_tensor("w2p", [128, 8 * 4096], F16, kind="ExternalInput")
    btp_d = nc.dram_tensor("btp", [128, n_blocks * 8 * R], F16, kind="ExternalInput")
    bfp_d = nc.dram_tensor("bfp", [128, n_blocks * RG * D_MODEL], F16,
                           kind="ExternalInput")
    b1_d = nc.dram_tensor("b1c", [128, M1], F32, kind="ExternalInput")
    b2_d = nc.dram_tensor("b2r", [128, D_MODEL], F32, kind="ExternalInput")
    out_d = nc.dram_tensor("out", [128, n_blocks * RG], F32, kind="ExternalOutput")

    n_groups = n_blocks * RG

    with tile.TileContext(nc) as tc, ExitStack() as ctx:
        wpool = ctx.enter_context(tc.tile_pool(name="weights", bufs=1))
        btpool = ctx.enter_context(tc.tile_pool(name="batchT", bufs=1))
        bfpool = ctx.enter_context(tc.tile_pool(name="batchf", bufs=1))
        ipool = ctx.enter_context(tc.tile_pool(name="innerT", bufs=36))
        wxpool = ctx.enter_context(tc.tile_pool(name="wx", bufs=4))
        spool = ctx.enter_context(tc.tile_pool(name="scratch", bufs=2))
        zpool = ctx.enter_context(tc.tile_pool(name="z", bufs=1))
        psum1 = ctx.enter_context(tc.tile_pool(name="psum1", bufs=3, space="PSUM"))
        psum2 = ctx.enter_context(tc.tile_pool(name="psum2", bufs=4, space="PSUM"))
        psumw = ctx.enter_context(tc.tile_pool(name="psumw", bufs=1, space="PSUM"))

        # --- small constants on the scalar issue queue (needed by ~12 us) ---
        b1t = wpool.tile([128, M1], F32, tag="b1t")
        nc.scalar.dma_start(b1t[:], b1_d.ap()[:])

        # --- PE warmup: zero matmuls during the initial DMA wait flip the
        # HAM clock gate to 8/8 and bridge until real data lands (~13 us) ---
        warm_s = wpool.tile([128, 128], F16, tag="warm_s")
        warm_m = wpool.tile([128, 512], F16, tag="warm_m")
        nc.vector.memset(warm_s[:], 0.0)
        nc.vector.memset(warm_m[:], 0.0)
        wps = psumw.tile([128, 512], F32)
        for _ in range(15):
            nc.tensor.matmul(wps[:], warm_s[:], warm_m[:], start=True, stop=True,
                             skip_group_check=True)

        # --- everything the critical path needs early goes on the SYNC queue
        # in exact consumption order (DMA engines round-robin across queue
        # ring-sets, so FIFO position within one queue is the only way to
        # prioritize): btp0, then W1 sixteenths, then W2 sixteenths with b2
        # slotted at its need time.  Later batch blocks go on the gpsimd
        # queue where bufs=1 buffer-reuse waits throttle them to exactly
        # when they're needed, keeping ring bandwidth on the weight stream ---
        bt_t, bf_t = [], []

        def emit_bt(b, eng):
            t = btpool.tile([128, 8 * R], F16, tag="bt")
            eng.dma_start(t[:], btp_d.ap()[:, b * 8 * R:(b + 1) * 8 * R])
            bt_t.append(t)

        def emit_bf(b, eng):
            f = bfpool.tile([128, RG * D_MODEL], F16, tag="bf")
            eng.dma_start(
                f[:], bfp_d.ap()[:, b * RG * D_MODEL:(b + 1) * RG * D_MODEL])
            bf_t.append(f)

        emit_bt(0, nc.sync)
        w1t = []
        for q in range(16):
            t = wpool.tile([128, 2048], F16, tag=f"w1q{q}")
            nc.sync.dma_start(t[:], w1p_d.ap()[:, q * 2048:(q + 1) * 2048])
            w1t.append(t)
        w2t = []
        b2r = None
        for q in range(16):
            if q == 8:
                b2r = wpool.tile([128, D_MODEL], F32, tag="b2r")
                nc.sync.dma_start(b2r[:], b2_d.ap()[:])
            t = wpool.tile([128, 2048], F16, tag=f"w2q{q}")
            nc.sync.dma_start(t[:], w2p_d.ap()[:, q * 2048:(q + 1) * 2048])
            w2t.append(t)

        # later batch blocks: btp1 first (its bufs=1 wait gates the rest of
        # the gpsimd queue until block-0 phase 1 has consumed btp0)
        if n_blocks > 1:
            emit_bt(1, nc.gpsimd)
        emit_bf(0, nc.gpsimd)
        if n_blocks > 1:
            emit_bf(1, nc.gpsimd)
        for b in range(2, n_blocks):
            emit_bt(b, nc.gpsimd)
            emit_bf(b, nc.gpsimd)

        def w1s(m, k):
            # stationary [128, 128] for phase-1 (m, k)
            q, mo2 = divmod(m, 2)
            off = (mo2 * 8 + k) * 128
            return w1t[q][:, off:off + 128]

        def w2s(m, h):
            # moving [128, 512] for phase-2 (m, h)
            q, ml = divmod(m, 2)
            off = ml * 1024 + h * 512
            return w2t[q][:, off:off + 512]

        # per-half dot partials: column h*n_groups+g holds sum over the h-th
        # 512 of d_model; summed pairwise at the end (shortens the tail chain)
        z_h = zpool.tile([128, 2 * n_groups], F32, tag="zh")
        z_all = zpool.tile([128, n_groups], F32)
        sig = zpool.tile([128, n_groups], F32, tag="sig")

        for b in range(n_blocks):
            bt = bt_t[b]
            # phase 1: innerT chunks [128 dff, R rows]
            it = []
            for m in range(M1):
                ps = psum1.tile([128, R], F32)
                for k in range(K1):
                    nc.tensor.matmul(
                        ps[:],
                        w1s(m, k),
                        bt[:, k * R:(k + 1) * R],
                        start=(k == 0),
                        stop=(k == K1 - 1),
                    )
                t = ipool.tile([128, R], F16, tag="it")
                nc.scalar.activation(
                    t[:], ps[:], mybir.ActivationFunctionType.Tanh,
                    bias=b1t[:, m:m + 1],
                )
                it.append(t)

            # phase 2 + row-dot per 128-row group
            for rg in range(RG):
                g = b * RG + rg
                wx = wxpool.tile([128, D_MODEL], F16, tag="wx")
                for h in range(NH):
                    ps2 = psum2.tile([128, 512], F32)
                    for m in range(M1):
                        nc.tensor.matmul(
                            ps2[:],
                            it[m][:, rg * 128:(rg + 1) * 128],
                            w2s(m, h),
                            start=(m == 0),
                            stop=(m == M1 - 1),
                        )
                    # b2 (free-dim bias): DVE add in-place on PSUM
                    nc.vector.tensor_tensor(
                        ps2[:], ps2[:], b2r[:, h * 512:(h + 1) * 512],
                        mybir.AluOpType.add,
                    )
                    nc.scalar.activation(
                        wx[:, h * 512:(h + 1) * 512], ps2[:],
                        mybir.ActivationFunctionType.Tanh,
                    )
                    # z_h = sum(wx * batch) over this 512 of d_model (DVE)
                    scratch = spool.tile([128, 512], F16, tag="scr")
                    nc.vector.scalar_tensor_tensor(
                        out=scratch[:],
                        in0=wx[:, h * 512:(h + 1) * 512],
                        scalar=1.0,
                        in1=bf_t[b][:, rg * D_MODEL + h * 512:
                                    rg * D_MODEL + (h + 1) * 512],
                        op0=mybir.AluOpType.mult,
                        op1=mybir.AluOpType.mult,
                        accum_out=z_h[:, h * n_groups + g:h * n_groups + g + 1],
                    )

        # z = z_h0 + z_h1; one sigmoid + one output DMA (host untransposes)
        nc.vector.tensor_tensor(
            z_all[:], z_h[:, 0:n_groups], z_h[:, n_groups:2 * n_groups],
            mybir.AluOpType.add,
        )
        nc.scalar.activation(
            sig[:], z_all[:], mybir.ActivationFunctionType.Sigmoid,
        )
        nc.sync.dma_start(out_d.ap()[:], sig[:])

    return nc


_CACHED = {}


def _get_nc(n_blocks=N_BLOCKS):
    if n_blocks not in _CACHED:
        _CACHED[n_blocks] = build_bass(n_blocks)
    return _CACHED[n_blocks]


def _prep_in_maps(batch, W1, b1, W2, b2):
    batch = np.ascontiguousarray(batch, dtype=np.float32)
    w1t = W1.T.astype(np.float16)                           # [1024, 4096]
    w2t = W2.T.astype(np.float16)                           # [4096, 1024]

    # w1p: [p, q, mo2, k, j] with m = q*2 + mo2
    #   A[k, p, m, j] -> [p, m(=32), k, j] -> split m into (16, 2) -> pack
    A = w1t.reshape(K1, 128, M1, 128).transpose(1, 2, 0, 3)   # [p, m, k, j]
    w1p = np.ascontiguousarray(
        A.reshape(128, 16, 2, K1, 128).reshape(128, 8 * 4096))

    # w2p: [p, q, ml, c] with m = q*2 + ml
    C = w2t.reshape(M1, 128, D_MODEL).transpose(1, 0, 2)      # [p, m, c]
    w2p = np.ascontiguousarray(C.reshape(128, 8 * 4096))
    # (m-major layout is identical for any even split; slicing handles q)

    # b1 as [128, 32]: column m holds b1[m*128:(m+1)*128] (per-partition bias)
    b1c = np.ascontiguousarray(
        np.asarray(b1, dtype=np.float32).reshape(M1, 128).T)
    # b2 replicated across partitions for the DVE free-dim bias add
    b2r = np.ascontiguousarray(
        np.broadcast_to(np.asarray(b2, dtype=np.float32)[None, :],
                        (128, D_MODEL)))

    batcht = batch.T.astype(np.float16)                       # [1024, 16384]
    batch16 = batch.astype(np.float16)                        # [16384, 1024]

    in_maps = []
    for c in range(N_CORES):
        r0, r1 = c * NC_ROWS, (c + 1) * NC_ROWS
        # btp: [p, b, k, r]
        D = batcht[:, r0:r1].reshape(K1, 128, N_BLOCKS, R).transpose(1, 2, 0, 3)
        btp = np.ascontiguousarray(D.reshape(128, N_BLOCKS * 8 * R))
        # bfp: [p, g, c]
        E = batch16[r0:r1].reshape(N_GROUPS, 128, D_MODEL).transpose(1, 0, 2)
        bfp = np.ascontiguousarray(E.reshape(128, N_GROUPS * D_MODEL))
        in_maps.append({
            "w1p": w1p,
            "w2p": w2p,
            "b1c": b1c,
            "b2r": b2r,
            "btp": btp,
            "bfp": bfp,
        })
    return in_maps


def kernel(batch, W1, b1, W2, b2, _trace=False, _trace_kwargs=None):
    in_maps = _prep_in_maps(batch, W1, b1, W2, b2)
    nc = _get_nc()
    res = bass_utils.run_bass_kernel_spmd(
        nc, in_maps, core_ids=list(range(N_CORES)),
        trace=_trace, **(_trace_kwargs or {}),
    )
    # out[p, g] holds row g*128+p of the core's 2048 rows
    out = np.concatenate([
        np.ascontiguousarray(res.results[c]["out"].T).reshape(-1)
        for c in range(N_CORES)
    ])
    if _trace:
        return out, res
    return out


# revision 17
# speedup vs baseline: 1.1052x; 1.0037x over previous
"""Trainium2 kernel for nn_MiddleHeadLayer: 2-layer tanh MLP + row-dot + sigmoid.

    inner = tanh(batch @ W1.T + b1)        batch [N, 1024], W1 [4096, 1024]
    wx    = tanh(inner @ W2.T + b2)        W2 [1024, 4096]
    out   = sigmoid(sum(wx * batch, -1))   [N]

Data-parallel over 8 NeuronCores: each core handles N/8 = 2048 rows;
weights replicated, resident in SBUF as fp16 (fp16 matmuls run at full PE
rate; absmax error ~4e-3, well inside the 2e-2 gate).

Per-core dataflow, in blocks of R=256 rows:
  phase 1: innerT[dff, rows] = tanh(W1T.T @ batchT + b1) — stationary W1T
           chunks [128,128], moving batchT [128, R], fp16 in / f32 PSUM,
           ACT applies the per-partition (d_ff) bias and writes fp16.
  phase 2: wx[rows, dmodel] = tanh(innerT.T @ W2T + b2) — stationary innerT
           chunks, moving W2T [128, 512]. b2 (free-dim bias) is added by a
           DVE tensor_tensor in-place on PSUM (saves 32 rank-1 PE matmuls).
  dot:     z[rows] = sum(wx * batch_f16) via fused DVE scalar_tensor_tensor
           along the free dim; ONE sigmoid + ONE output DMA at the end.

DMA strategy (the baseline's bottleneck): all inputs are pre-packed on the
host into [128, L]-shaped tensors whose per-partition lines are 4-8 KB, so
the whole kernel needs ~28 large DMAs instead of 163 small ones.  Each
dma_start costs ~650 ns of issue time on its engine queue, so issue is
split across three queues (sync: W1/W2 + output, gpsimd: batchT/batch,
scalar: small constants) and ordered just-in-time so the PE's first matmul
can start ~11 us in and never starves.  Eight zero-input warmup matmuls run
during the initial DMA wait to flip the PE HAM clock gate to full rate
before real work arrives.
"""

from contextlib import ExitStack

import numpy as np
import orjson

import concourse.bass as bass
import concourse.tile as tile
from concourse import mybir
from concourse import bass_utils

D_MODEL = 1024
D_FF = 4096
N_TOTAL = 16384
N_CORES = 8
NC_ROWS = N_TOTAL // N_CORES          # 2048 rows per core
R = 256                               # row-block size
N_BLOCKS = NC_ROWS // R               # 8
K1 = D_MODEL // 128                   # 8 contraction chunks for matmul1
M1 = D_FF // 128                      # 32 d_ff chunks
RG = R // 128                         # row groups per block (2)
NH = D_MODEL // 512                   # d_model halves for phase 2 (2)
N_GROUPS = N_BLOCKS * RG              # 16
F16 = mybir.dt.float16
F32 = mybir.dt.float32


# ---------------------------------------------------------------------------
# This walrus build rejects >2 sem waits on a single instruction, while Tile's
# wait assignment freely attaches more (e.g. the exit drain gets one wait per
# outstanding logical proc). Legalize at the BIR-JSON level: hoist excess
# waits onto EventSemaphore instructions inserted directly before the
# offending instruction on the same engine stream (identical semantics).
MAX_WAITS = 1


def _legalize_sync_waits(bir: dict) -> dict:
    ctr = 0
    for fn in bir.get("functions", []):
        for blk in fn.get("blocks", []):
            insts = blk.get("instructions")
            if not insts:
                continue
            out = []
            changed = False
            for inst in insts:
                si = inst.get("sync_info")
                ow = (si or {}).get("on_wait") or []
                limit = 2 if inst.get("opcode") == "EventSemaphore" else MAX_WAITS
                if len(ow) > limit:
                    changed = True
                    excess, keep = ow[:-limit], ow[-limit:]
                    for i in range(0, len(excess), 2):
                        ctr += 1
                        out.append({
                            "debug": inst.get("debug"),
                            "engine": inst["engine"],
                            "ins": [],
                            "outs": [],
                            "name": f"legalwait-{ctr}",
                            "opcode": "EventSemaphore",
                            "sync_info": {
                                "on_update": [],
                                "on_wait": excess[i:i + 2],
                            },
                        })
                    si["on_wait"] = keep
                out.append(inst)
            if changed:
                blk["instructions"] = out
    return bir


_orig_to_json_bytes = bass.Bass.to_json_bytes


def _patched_to_json_bytes(self) -> bytes:
    return orjson.dumps(_legalize_sync_waits(orjson.loads(_orig_to_json_bytes(self))))


bass.Bass.to_json_bytes = _patched_to_json_bytes


def build_bass(n_blocks=N_BLOCKS):
    nc = bass.Bass("TRN2", target_bir_lowering=False, debug=False)

    # Packed DRAM layouts (see _prep_in_maps for the exact packing):
    #  w1p[p, (q*16 + mo2*8 + k)*128 + j] = W1T[k*128+p, (q*2+mo2)*128 + j]
    #  w2p[p, (q*4 + ml)*1024 + c]         = W2T[(q*4+ml)*128 + p, c]
    #  btp[p, (b*8 + k)*256 + r]           = batchT[k*128+p, b*256 + r]
    #  bfp[p, g*1024 + c]                  = batch[g*128+p, c]            (fp16)
    w1p_d = nc.dram_tensor("w1p", [128, 8 * 4096], F16, kind="ExternalInput")
    w2p_d = nc.dram_tensor("w2p", [128, 8 * 4096], F16, kind="ExternalInput")
    btp_d = nc.dram_tensor("btp", [128, n_blocks * 8 * R], F16, kind="ExternalInput")
    bfp_d = nc.dram_tensor("bfp", [128, n_blocks * RG * D_MODEL], F16,
                           kind="ExternalInput")
    b1_d = nc.dram_tensor("b1c", [128, M1], F32, kind="ExternalInput")
    b2_d = nc.dram_tensor("b2r", [128, D_MODEL], F32, kind="ExternalInput")
    out_d = nc.dram_tensor("out", [128, n_blocks * RG], F32, kind="ExternalOutput")

    n_groups = n_blocks * RG

    with tile.TileContext(nc) as tc, ExitStack() as ctx:
        wpool = ctx.enter_context(tc.tile_pool(name="weights", bufs=1))
        btpool = ctx.enter_context(tc.tile_pool(name="batchT", bufs=1))
        bfpool = ctx.enter_context(tc.tile_pool(name="batchf", bufs=1))
        ipool = ctx.enter_context(tc.tile_pool(name="innerT", bufs=36))
        wxpool = ctx.enter_context(tc.tile_pool(name="wx", bufs=4))
        spool = ctx.enter_context(tc.tile_pool(name="scratch", bufs=2))
        zpool = ctx.enter_context(tc.tile_pool(name="z", bufs=1))
        psum1 = ctx.enter_context(tc.tile_pool(name="psum1", bufs=3, space="PSUM"))
        psum2 = ctx.enter_context(tc.tile_pool(name="psum2", bufs=4, space="PSUM"))
        psumw = ctx.enter_context(tc.tile_pool(name="psumw", bufs=1, space="PSUM"))

        # --- small constants on the scalar issue queue (needed by ~12 us) ---
        b1t = wpool.tile([128, M1], F32, tag="b1t")
        nc.scalar.dma_start(b1t[:], b1_d.ap()[:])

        # --- PE warmup: zero matmuls during the initial DMA wait flip the
        # HAM clock gate to 8/8 and bridge until real data lands (~13 us) ---
        warm_s = wpool.tile([128, 128], F16, tag="warm_s")
        warm_m = wpool.tile([128, 512], F16, tag="warm_m")
        nc.vector.memset(warm_s[:], 0.0)
        nc.vector.memset(warm_m[:], 0.0)
        wps = psumw.tile([128, 512], F32)
        for _ in range(14):
            nc.tensor.matmul(wps[:], warm_s[:], warm_m[:], start=True, stop=True,
                             skip_group_check=True)

        # --- everything the critical path needs early goes on the SYNC queue
        # in exact consumption order (DMA engines round-robin across queue
        # ring-sets, so FIFO position within one queue is the only way to
        # prioritize): btp0, then W1 sixteenths, then W2 sixteenths with b2
        # slotted at its need time.  Later batch blocks go on the gpsimd
        # queue where bufs=1 buffer-reuse waits throttle them to exactly
        # when they're needed, keeping ring bandwidth on the weight stream ---
        bt_t, bf_t = [], []

        def emit_bt(b, eng):
            t = btpool.tile([128, 8 * R], F16, tag="bt")
            eng.dma_start(t[:], btp_d.ap()[:, b * 8 * R:(b + 1) * 8 * R])
            bt_t.append(t)

        def emit_bf(b, eng):
            f = bfpool.tile([128, RG * D_MODEL], F16, tag="bf")
            eng.dma_start(
                f[:], bfp_d.ap()[:, b * RG * D_MODEL:(b + 1) * RG * D_MODEL])
            bf_t.append(f)

        emit_bt(0, nc.sync)
        w1t = []
        for q in range(16):
            t = wpool.tile([128, 2048], F16, tag=f"w1q{q}")
            nc.sync.dma_start(t[:], w1p_d.ap()[:, q * 2048:(q + 1) * 2048])
            w1t.append(t)
        w2t = []
        b2r = None
        for q in range(16):
            if q == 4:
                b2r = wpool.tile([128, D_MODEL], F32, tag="b2r")
                nc.sync.dma_start(b2r[:], b2_d.ap()[:])
            if q == 6:
                # block-0 dot operand rides the sync FIFO at its need slot
                # (on gpsimd, Tile's readiness-order would run it too early)
                emit_bf(0, nc.sync)
            t = wpool.tile([128, 2048], F16, tag=f"w2q{q}")
            nc.sync.dma_start(t[:], w2p_d.ap()[:, q * 2048:(q + 1) * 2048])
            w2t.append(t)
        if n_blocks > 1:
            emit_bf(1, nc.sync)

        # later batch blocks on the gpsimd queue: every entry carries a
        # bufs=1 buffer-reuse wait, so they self-throttle to exactly when
        # they're needed and never compete with the critical weight stream
        if n_blocks > 1:
            emit_bt(1, nc.gpsimd)
        for b in range(2, n_blocks):
            emit_bt(b, nc.gpsimd)
            emit_bf(b, nc.gpsimd)

        def w1s(m, k):
            # stationary [128, 128] for phase-1 (m, k)
            q, mo2 = divmod(m, 2)
            off = (mo2 * 8 + k) * 128
            return w1t[q][:, off:off + 128]

        def w2s(m, h):
            # moving [128, 512] for phase-2 (m, h)
            q, ml = divmod(m, 2)
            off = ml * 1024 + h * 512
            return w2t[q][:, off:off + 512]

        # per-half dot partials: column h*n_groups+g holds sum over the h-th
        # 512 of d_model; summed pairwise at the end (shortens the tail chain)
        z_h = zpool.tile([128, 2 * n_groups], F32, tag="zh")
        z_all = zpool.tile([128, n_groups], F32)
        sig = zpool.tile([128, n_groups], F32, tag="sig")

        for b in range(n_blocks):
            bt = bt_t[b]
            # phase 1: innerT chunks [128 dff, R rows]
            it = []
            for m in range(M1):
                ps = psum1.tile([128, R], F32)
                for k in range(K1):
                    nc.tensor.matmul(
                        ps[:],
                        w1s(m, k),
                        bt[:, k * R:(k + 1) * R],
                        start=(k == 0),
                        stop=(k == K1 - 1),
                    )
                t = ipool.tile([128, R], F16, tag="it")
                nc.scalar.activation(
                    t[:], ps[:], mybir.ActivationFunctionType.Tanh,
                    bias=b1t[:, m:m + 1],
                )
                it.append(t)

            # phase 2 + row-dot per 128-row group
            for rg in range(RG):
                g = b * RG + rg
                wx = wxpool.tile([128, D_MODEL], F16, tag="wx")
                for h in range(NH):
                    ps2 = psum2.tile([128, 512], F32)
                    for m in range(M1):
                        nc.tensor.matmul(
                            ps2[:],
                            it[m][:, rg * 128:(rg + 1) * 128],
                            w2s(m, h),
                            start=(m == 0),
                            stop=(m == M1 - 1),
                        )
                    # b2 (free-dim bias): DVE add in-place on PSUM
                    nc.vector.tensor_tensor(
                        ps2[:], ps2[:], b2r[:, h * 512:(h + 1) * 512],
                        mybir.AluOpType.add,
                    )
                    nc.scalar.activation(
                        wx[:, h * 512:(h + 1) * 512], ps2[:],
                        mybir.ActivationFunctionType.Tanh,
                    )
                    # z_h = sum(wx * batch) over this 512 of d_model (DVE)
                    scratch = spool.tile([128, 512], F16, tag="scr")
                    nc.vector.scalar_tensor_tensor(
                        out=scratch[:],
                        in0=wx[:, h * 512:(h + 1) * 512],
                        scalar=1.0,
                        in1=bf_t[b][:, rg * D_MODEL + h * 512:
                                    rg * D_MODEL + (h + 1) * 512],
                        op0=mybir.AluOpType.mult,
                        op1=mybir.AluOpType.mult,
                        accum_out=z_h[:, h * n_groups + g:h * n_groups + g + 1],
                    )

        # z = z_h0 + z_h1; one sigmoid + one output DMA (host untransposes)
        nc.vector.tensor_tensor(
            z_all[:], z_h[:, 0:n_groups], z_h[:, n_groups:2 * n_groups],
            mybir.AluOpType.add,
        )
        nc.scalar.activation(
            sig[:], z_all[:], mybir.ActivationFunctionType.Sigmoid,
        )
        nc.sync.dma_start(out_d.ap()[:], sig[:])

    return nc


_CACHED = {}


def _get_nc(n_blocks=N_BLOCKS):
    if n_blocks not in _CACHED:
        _CACHED[n_blocks] = build_bass(n_blocks)
    return _CACHED[n_blocks]


def _prep_in_maps(batch, W1, b1, W2, b2):
    batch = np.ascontiguousarray(batch, dtype=np.float32)
    w1t = W1.T.astype(np.float16)                           # [1024, 4096]
    w2t = W2.T.astype(np.float16)                           # [4096, 1024]

    # w1p: [p, q, mo2, k, j] with m = q*2 + mo2
    #   A[k, p, m, j] -> [p, m(=32), k, j] -> split m into (16, 2) -> pack
    A = w1t.reshape(K1, 128, M1, 128).transpose(1, 2, 0, 3)   # [p, m, k, j]
    w1p = np.ascontiguousarray(
        A.reshape(128, 16, 2, K1, 128).reshape(128, 8 * 4096))

    # w2p: [p, q, ml, c] with m = q*2 + ml
    C = w2t.reshape(M1, 128, D_MODEL).transpose(1, 0, 2)      # [p, m, c]
    w2p = np.ascontiguousarray(C.reshape(128, 8 * 4096))
    # (m-major layout is identical for any even split; slicing handles q)

    # b1 as [128, 32]: column m holds b1[m*128:(m+1)*128] (per-partition bias)
    b1c = np.ascontiguousarray(
        np.asarray(b1, dtype=np.float32).reshape(M1, 128).T)
    # b2 replicated across partitions for the DVE free-dim bias add
    b2r = np.ascontiguousarray(
        np.broadcast_to(np.asarray(b2, dtype=np.float32)[None, :],
                        (128, D_MODEL)))

    batcht = batch.T.astype(np.float16)                       # [1024, 16384]
    batch16 = batch.astype(np.float16)                        # [16384, 1024]

    in_maps = []
    for c in range(N_CORES):
        r0, r1 = c * NC_ROWS, (c + 1) * NC_ROWS
        # btp: [p, b, k, r]
        D = batcht[:, r0:r1].reshape(K1, 128, N_BLOCKS, R).transpose(1, 2, 0, 3)
        btp = np.ascontiguousarray(D.reshape(128, N_BLOCKS * 8 * R))
        # bfp: [p, g, c]
        E = batch16[r0:r1].reshape(N_GROUPS, 128, D_MODEL).transpose(1, 0, 2)
        bfp = np.ascontiguousarray(E.reshape(128, N_GROUPS * D_MODEL))
        in_maps.append({
            "w1p": w1p,
            "w2p": w2p,
            "b1c": b1c,
            "b2r": b2r,
            "btp": btp,
            "bfp": bfp,
        })
    return in_maps


def kernel(batch, W1, b1, W2, b2, _trace=False, _trace_kwargs=None):
    in_maps = _prep_in_maps(batch, W1, b1, W2, b2)
    nc = _get_nc()
    res = bass_utils.run_bass_kernel_spmd(
        nc, in_maps, core_ids=list(range(N_CORES)),
        trace=_trace, **(_trace_kwargs or {}),
    )
    # out[p, g] holds row g*128+p of the core's 2048 rows
    out = np.concatenate([
        np.ascontiguousarray(res.results[c]["out"].T).reshape(-1)
        for c in range(N_CORES)
    ])
    if _trace:
        return out, res
    return out
